# revision 29
# baseline (speedup 1.0000x reference)
"""Trainium2 Bass kernel for nn_EnhancedAttentionLayer (B=4, S=2048, D=1024).

Single-head attention, fp32 in/out. Sharding: 8 cores = (batch b in 0..3) x
(query-half h in 0..1); each core produces the output rows for its 1024
queries. Two algebraic restructurings cut PE work ~20% vs the direct
Q/K/V/scores/ctx/out pipeline (1056 vs 1312 128x128x512-equivalent matmuls
per core), with no cross-core traffic:

  M-trick   scores = x^T (Wq^T Wk) x. Compute M = Wq^T Wk (128 MMs, weights
            only) and Zq = M^T xq (128), replacing Q-proj (128) + K-proj
            (256): K is never materialized.
  Late-V    ctx^T = Wv^T (xn^T expT) and y = Wo ctx, folded: precompute
            W2T = (Wo Wv)^T as Wv^T Wo^T (128 MMs, weights only), then
            G = xn^T @ expT (256) and y = W2T^T @ Gn (128), replacing
            V-proj (256) + ctx (256) + out-proj (128). The attention
            contraction (2048 keys) is applied to raw x BEFORE any weight
            projection, so projections only ever see 1024 columns.

Key-roll: each core's xt/xn have the key axis rotated so its own 1024 query
positions come first (attention is key-order invariant); xq is then always
xt cols 0:1024 -- no separate xq input, and the schedule is SPMD-identical
across cores.

All matmul operands are bf16 (PSUM accumulates fp32): same PE rate as
fp32r at 512-moving, but LDWEIGHTS runs FWL (~107ns vs 187ns fp32-HIGH),
which is what held fp32r chains at ~227ns/MM instead of 216. Intermediates
(M, Zq, W2T, expT, G) are drained from PSUM straight to bf16. Measured
error vs the fp32 reference is ~8e-3 max-normalized (gate 2e-2); the
fp32r variant of this same kernel measured 3.45e-3 at +7us.

Phases: warmup (40 dummy matmuls on ones while the first weight chunks
stream in, so the HAM throttle hits K=8/8 before the first real MM),
  A:  M = wq^T wk ; Zq = M^T @ xq ; W2T = wv^T wot
  B1: ST[k,q] = xt^T @ Zq ; expT = exp(ST/32) -> bf16 (ACT, fused scale).
      The DVE keeps exact f32 running colsums of the exp chunks; the
      cross-partition sum is then 2 fp32 ones-matmuls (one per q-half,
      replicating the per-query sum on all 128 partitions -> full-width
      reciprocal) instead of 32 bf16 ones-matmuls. Those 2 matmuls are
      deferred past the first G chain so the PE never waits on the DVE
      add tail.
  B2: G[d,q] = xn^T @ expT ; Gn = G * recip (DVE, PSUM drain)
      ytT[o,q] = W2T^T @ Gn -> DRAM
512-moving matmuls through a 6-bank PSUM rotation (+2 banks for the
colsum matmuls); 1024-moving fails walrus codegen (s3d3_mm_num_elements).
Weight loads are split across the Sync and Scalar HWDGE queues so chunk
pairs land 2x faster; wv/wo reuse wq/wk's SBUF bytes (zone-gated on M's
last matmul, landing before W2T needs them). Measured: 240549ns HW exec
(vs 327390ns baseline), rel err 7.32e-3. Runs on a thermally-throttled
chip (P0, PE at 2.0GHz instead of 2.4) measure ~18% slower.
Biases are zeros by problem spec; bo is applied on host if nonzero.
"""
import sys

if '/opt/trn_rl_repo' not in sys.path:
    sys.path.insert(0, '/opt/trn_rl_repo')

from contextlib import ExitStack

import numpy as np
import ml_dtypes

import concourse.bacc as bacc_mod
import concourse.mybir as mybir
import concourse.tile as tile
from concourse.bass_utils import run_bass_kernel_spmd

F32 = mybir.dt.float32
BF16 = mybir.dt.bfloat16
EXP = mybir.ActivationFunctionType.Exp
MULT = mybir.AluOpType.mult
ADD = mybir.AluOpType.add

B, S, D = 4, 2048, 1024
SQ = 1024           # queries per core
P = 128
NDC = D // P        # 8 chunks of 128 over d / d' / e / o
NKC = S // P        # 16 key chunks
NQH = SQ // 512     # 2 query column-halves (moving dim 512)
NH2 = D // 512      # 2 column-halves of a [*, D] product

LAST_RESULT = [None]
_CACHE = {}


def build_nc():
    nc = bacc_mod.Bacc("TRN2", target_bir_lowering=False, debug=False)

    xt = nc.dram_tensor("xt", [D, S], BF16, kind="ExternalInput")
    xn = nc.dram_tensor("xn", [S, D], BF16, kind="ExternalInput")
    wq = nc.dram_tensor("wq", [D, D], BF16, kind="ExternalInput")
    wk = nc.dram_tensor("wk", [D, D], BF16, kind="ExternalInput")
    wv = nc.dram_tensor("wv", [D, D], BF16, kind="ExternalInput")
    wot = nc.dram_tensor("wot", [D, D], BF16, kind="ExternalInput")
    yt = nc.dram_tensor("yt", [D, SQ], F32, kind="ExternalOutput")

    def part3(ap):  # [R, C] dram -> [128, R/128, C] (rows on partitions)
        return ap.rearrange("(o i) c -> i o c", i=P)

    with tile.TileContext(nc) as tc, ExitStack() as ctx:
        pers = ctx.enter_context(tc.tile_pool(name="pers", bufs=1))
        ones_bf = pers.tile([P, P], BF16)
        nc.vector.memset(ones_bf[:], 1.0)
        bcast_sb = pers.tile([P, SQ], F32)
        acc_sb = pers.tile([P, SQ], F32)   # per-partition partial colsums
        accb_sb = pers.tile([P, 512], BF16)  # bf16 cast for the ones-matmul

        # 6-bank PSUM rotation shared by every accumulation chain; +2 banks
        # for the two q-halves' colsum accumulators during B1
        mps = ctx.enter_context(tc.tile_pool(name="mps", bufs=6, space="PSUM"))

        # persistents: W2T, x^T, Zq, G (left); expT, xn (right)
        w2tp = ctx.enter_context(tc.tile_pool(name="w2t", bufs=1))
        w2t_sb = w2tp.tile([P, NDC, D], BF16)      # 16 KB/part
        xtp = ctx.enter_context(tc.tile_pool(name="xtp", bufs=1))
        xt_sb = xtp.tile([P, NDC, S], BF16)        # 32 KB/part
        zqp = ctx.enter_context(tc.tile_pool(name="zqp", bufs=1))
        zq_sb = zqp.tile([P, NDC, SQ], BF16)       # 16 KB/part
        gp = ctx.enter_context(tc.tile_pool(name="gp", bufs=1))
        g_sb = gp.tile([P, NDC, SQ], BF16)         # 16 KB/part
        yp = ctx.enter_context(tc.tile_pool(name="yp", bufs=3))
        epool = ctx.enter_context(
            tc.tile_pool(name="expt", bufs=1, side="right"))
        expt_sb = epool.tile([P, NKC, SQ], BF16)   # 32 KB/part
        xnp = ctx.enter_context(
            tc.tile_pool(name="xnp", bufs=1, side="right"))
        xn_sb = xnp.tile([P, NKC, D], BF16)        # 32 KB/part

        # ~4.3us of dummy matmuls on ones_bf while the first weight chunks
        # stream in: keeps the PE busy from ~8us so the HAM throttle reaches
        # K=8/8 before the first real matmul
        ps_w = mps.tile([P, 512], F32, tag="ps", name="warm")
        for i in range(40):
            nc.tensor.matmul(ps_w[:, 0:P], ones_bf[:], ones_bf[:],
                             start=(i == 0), stop=(i == 39))
        nc.vector.tensor_copy(acc_sb[:, 0:P], ps_w[:, 0:P])  # dead store

        def chain_waves(chains, lhs_of, rhs_of, out_of, nacc, tagbase):
            # waves of 6 chains; acc-step outer so 6 independent PSUM
            # accumulations ride out chunked-DMA arrival
            for w0 in range(0, len(chains), 6):
                wave = chains[w0:w0 + 6]
                ps = [mps.tile([P, 512], F32, tag="ps",
                               name=f"{tagbase}{w0}_{i}")
                      for i in range(len(wave))]
                for a in range(nacc):
                    for i, ch in enumerate(wave):
                        nc.tensor.matmul(ps[i][:], lhs_of(ch, a),
                                         rhs_of(ch, a),
                                         start=(a == 0), stop=(a == nacc - 1))
                for i, ch in enumerate(wave):
                    nc.vector.tensor_copy(out_of(ch), ps[i][:])

        chains = [(dc, h2) for dc in range(NDC) for h2 in range(NH2)]

        with tc.tile_pool(name="mp", bufs=1) as mp:
            m_sb = mp.tile([P, NDC, D], BF16)      # 16 KB/part
            # ---- A1: M = Wq^T Wk ----
            with tc.tile_pool(name="wqk", bufs=1) as wqk:
                wq_sb = wqk.tile([P, NDC, D], BF16)
                wk_sb = wqk.tile([P, NDC, D], BF16)
                # wq on sync, wk on scalar so chunk pairs land 2x faster;
                # chunk 0 split in half-columns so the first matmul's
                # operands arrive ~0.5us earlier
                for h in range(2):
                    nc.sync.dma_start(wq_sb[:, 0, h * 512:(h + 1) * 512],
                                      wq[0:P, h * 512:(h + 1) * 512])
                    nc.scalar.dma_start(wk_sb[:, 0, h * 512:(h + 1) * 512],
                                        wk[0:P, h * 512:(h + 1) * 512])
                for c in range(1, NDC):
                    nc.sync.dma_start(wq_sb[:, c, :],
                                      wq[c * P:(c + 1) * P, :])
                    nc.scalar.dma_start(wk_sb[:, c, :],
                                        wk[c * P:(c + 1) * P, :])
                # x^T and xn stream behind the weights on both queues
                for sh in range(2):
                    nc.sync.dma_start(
                        xt_sb[:, :, sh * 1024:sh * 1024 + 512],
                        part3(xt[:, sh * 1024:sh * 1024 + 512]))
                    nc.scalar.dma_start(
                        xt_sb[:, :, sh * 1024 + 512:(sh + 1) * 1024],
                        part3(xt[:, sh * 1024 + 512:(sh + 1) * 1024]))
                nc.sync.dma_start(xn_sb[:], part3(xn))
                # M[d, d'] = sum_e1 Wq[e1, d] Wk[e1, d']
                chain_waves(
                    chains,
                    lambda ch, a: wq_sb[:, a, ch[0] * P:(ch[0] + 1) * P],
                    lambda ch, a: wk_sb[:, a, ch[1] * 512:(ch[1] + 1) * 512],
                    lambda ch: m_sb[:, ch[0], ch[1] * 512:(ch[1] + 1) * 512],
                    NDC, "mm")

            # ---- A2: Zq = M^T @ xq (xq = xt cols 0:1024) ----
            chain_waves(
                chains,
                lambda ch, a: m_sb[:, a, ch[0] * P:(ch[0] + 1) * P],
                lambda ch, a: xt_sb[:, a, ch[1] * 512:(ch[1] + 1) * 512],
                lambda ch: zq_sb[:, ch[0], ch[1] * 512:(ch[1] + 1) * 512],
                NDC, "zq")

            # ---- A3: W2T = Wv^T Wo^T (wv/wo reuse wq/wk's bytes; DMA is
            # zone-gated on M's last matmul, landing before W2T starts) ----
            with tc.tile_pool(name="wvo", bufs=1) as wvo:
                wv_sb = wvo.tile([P, NDC, D], BF16)
                wo_sb = wvo.tile([P, NDC, D], BF16)
                for c in range(NDC):
                    nc.sync.dma_start(wv_sb[:, c, :],
                                      wv[c * P:(c + 1) * P, :])
                    nc.scalar.dma_start(wo_sb[:, c, :],
                                        wot[c * P:(c + 1) * P, :])
                # W2T[d, o] = sum_e Wv[e, d] Wo[o, e]
                chain_waves(
                    chains,
                    lambda ch, a: wv_sb[:, a, ch[0] * P:(ch[0] + 1) * P],
                    lambda ch, a: wo_sb[:, a, ch[1] * 512:(ch[1] + 1) * 512],
                    lambda ch: w2t_sb[:, ch[0],
                                      ch[1] * 512:(ch[1] + 1) * 512],
                    NDC, "w2")

        # ---- B1: scoresT -> expT; DVE keeps per-partition running colsums
        # (f32, exact) so the PE does 2 fp32 ones-matmuls instead of 32 ----
        with tc.tile_pool(name="sump", bufs=2, space="PSUM") as sump:
            for qh in range(NQH):
                q0 = qh * 512
                for kc in range(NKC):
                    ps_s = mps.tile([P, 512], F32, tag="ps",
                                    name=f"pss{qh}_{kc}")
                    for dc in range(NDC):
                        nc.tensor.matmul(
                            ps_s[:], xt_sb[:, dc, kc * P:(kc + 1) * P],
                            zq_sb[:, dc, q0:q0 + 512],
                            start=(dc == 0), stop=(dc == NDC - 1))
                    nc.scalar.activation(
                        expt_sb[:, kc, q0:q0 + 512], ps_s[:], EXP,
                        scale=1.0 / 32.0)
                    if kc == 0:
                        nc.vector.tensor_copy(acc_sb[:, q0:q0 + 512],
                                              expt_sb[:, 0, q0:q0 + 512])
                    else:
                        nc.vector.tensor_tensor(
                            acc_sb[:, q0:q0 + 512], acc_sb[:, q0:q0 + 512],
                            expt_sb[:, kc, q0:q0 + 512], ADD)

            # first G chain runs before the colsum matmuls so the PE never
            # waits on the tail of the DVE add chain
            ps_g0 = mps.tile([P, 512], F32, tag="ps", name="pg0_0")
            for kc in range(NKC):
                nc.tensor.matmul(
                    ps_g0[:], xn_sb[:, kc, 0:P], expt_sb[:, kc, 0:512],
                    start=(kc == 0), stop=(kc == NKC - 1))
            # cross-partition sum + broadcast in one fp32 ones-matmul per
            # q-half -> full-width reciprocal
            for qh in range(NQH):
                q0 = qh * 512
                ps_sum = sump.tile([P, 512], F32, tag="pssum")
                nc.vector.tensor_copy(accb_sb[:],
                                      acc_sb[:, q0:q0 + 512])
                nc.tensor.matmul(ps_sum[:], ones_bf[:], accb_sb[:],
                                 start=True, stop=True)
                nc.vector.reciprocal(bcast_sb[:, q0:q0 + 512], ps_sum[:])
            nc.vector.tensor_tensor(
                g_sb[:, 0, 0:512], ps_g0[:], bcast_sb[:, 0:512], MULT)

        # ---- B2: G = xn^T @ expT, normalized; ytT = W2T^T @ Gn ----
        for qh in range(NQH):
            q0 = qh * 512
            for dc in range(NDC):
                if qh == 0 and dc == 0:
                    continue  # already issued above
                ps_g = mps.tile([P, 512], F32, tag="ps", name=f"pg{qh}_{dc}")
                for kc in range(NKC):
                    nc.tensor.matmul(
                        ps_g[:], xn_sb[:, kc, dc * P:(dc + 1) * P],
                        expt_sb[:, kc, q0:q0 + 512],
                        start=(kc == 0), stop=(kc == NKC - 1))
                nc.vector.tensor_tensor(
                    g_sb[:, dc, q0:q0 + 512], ps_g[:],
                    bcast_sb[:, q0:q0 + 512], MULT)
        for qh in range(NQH):
            q0 = qh * 512
            for oc in range(NDC):
                ps_y = mps.tile([P, 512], F32, tag="ps", name=f"py{qh}_{oc}")
                for dc in range(NDC):
                    nc.tensor.matmul(
                        ps_y[:], w2t_sb[:, dc, oc * P:(oc + 1) * P],
                        g_sb[:, dc, q0:q0 + 512],
                        start=(dc == 0), stop=(dc == NDC - 1))
                yst = yp.tile([P, 512], F32, tag="yst")
                if qh == NQH - 1 and oc == NDC - 1:
                    # final drain split in half so the last store starts
                    # while the second half-copy runs
                    for h in range(2):
                        h0 = h * 256
                        nc.vector.tensor_copy(yst[:, h0:h0 + 256],
                                              ps_y[:, h0:h0 + 256])
                        nc.scalar.dma_start(
                            yt[oc * P:(oc + 1) * P, q0 + h0:q0 + h0 + 256],
                            yst[:, h0:h0 + 256])
                else:
                    nc.vector.tensor_copy(yst[:], ps_y[:])
                    nc.scalar.dma_start(
                        yt[oc * P:(oc + 1) * P, q0:q0 + 512], yst[:])

    nc.compile()
    return nc


def _get_nc():
    if "nc" not in _CACHE:
        _CACHE["nc"] = build_nc()
    return _CACHE["nc"]


def kernel(x, Wq, bq, Wk, bk, Wv, bv, Wo, bo, _trace=False):
    x = np.asarray(x, dtype=np.float32)
    bf = ml_dtypes.bfloat16
    wq_b = np.ascontiguousarray(np.asarray(Wq, dtype=np.float32)).astype(bf)
    wk_b = np.ascontiguousarray(np.asarray(Wk, dtype=np.float32)).astype(bf)
    wv_b = np.ascontiguousarray(np.asarray(Wv, dtype=np.float32)).astype(bf)
    wot_b = np.ascontiguousarray(
        np.asarray(Wo, dtype=np.float32).T).astype(bf)

    in_maps = []
    for c in range(8):
        b, h = c // 2, c % 2
        xb = x[b]
        if h == 0:
            xroll = xb
        else:
            xroll = np.concatenate([xb[SQ:], xb[:SQ]], axis=0)
        xtb_ = np.ascontiguousarray(xroll.T).astype(bf)
        xnb = np.ascontiguousarray(xroll).astype(bf)
        in_maps.append({"xt": xtb_, "xn": xnb, "wq": wq_b, "wk": wk_b,
                        "wv": wv_b, "wot": wot_b})

    nc = _get_nc()
    kw = {}
    if _trace:
        kw = dict(trace=True, stitch_traces=False)
    res = run_bass_kernel_spmd(nc, in_maps, core_ids=list(range(8)), **kw)
    LAST_RESULT[0] = res

    y = np.empty((B, S, D), dtype=np.float32)
    for c in range(8):
        b, h = c // 2, c % 2
        y[b, h * SQ:(h + 1) * SQ, :] = res.results[c]["yt"].T

    bo = np.asarray(bo, dtype=np.float32)
    if bo.any():
        y = y + bo
    return y


# revision 32
# speedup vs baseline: 1.2151x; 1.2151x over previous
"""Trainium2 Bass kernel for nn_EnhancedAttentionLayer (B=4, S=2048, D=1024).

Single-head attention, fp32 in/out. Sharding: 8 cores = (batch b in 0..3) x
(query-half h in 0..1); each core produces the output rows for its 1024
queries. Two algebraic restructurings cut PE work ~20% vs the direct
Q/K/V/scores/ctx/out pipeline (1056 vs 1312 128x128x512-equivalent matmuls
per core), with no cross-core traffic:

  M-trick   scores = x^T (Wq^T Wk) x. Compute M = Wq^T Wk (128 MMs, weights
            only) and Zq = M^T xq (128), replacing Q-proj (128) + K-proj
            (256): K is never materialized.
  Late-V    ctx^T = Wv^T (xn^T expT) and y = Wo ctx, folded: precompute
            W2T = (Wo Wv)^T as Wv^T Wo^T (128 MMs, weights only), then
            G = xn^T @ expT (256) and y = W2T^T @ Gn (128), replacing
            V-proj (256) + ctx (256) + out-proj (128). The attention
            contraction (2048 keys) is applied to raw x BEFORE any weight
            projection, so projections only ever see 1024 columns.

Key-roll: each core's xt/xn have the key axis rotated so its own 1024 query
positions come first (attention is key-order invariant); xq is then always
xt cols 0:1024 -- no separate xq input, and the schedule is SPMD-identical
across cores.

All matmul operands are bf16 (PSUM accumulates fp32): same PE rate as
fp32r at 512-moving, but LDWEIGHTS runs FWL (~107ns vs 187ns fp32-HIGH),
which is what held fp32r chains at ~227ns/MM instead of 216. Intermediates
(M, Zq, W2T, expT, G) are drained from PSUM straight to bf16. Measured
error vs the fp32 reference is ~8e-3 max-normalized (gate 2e-2); the
fp32r variant of this same kernel measured 3.45e-3 at +7us.

Phases: warmup (40 dummy matmuls on ones while the first weight chunks
stream in, so the HAM throttle hits K=8/8 before the first real MM),
  A:  M = wq^T wk ; Zq = M^T @ xq ; W2T = wv^T wot
  B1: ST[k,q] = xt^T @ Zq ; expT = exp(ST/32) -> bf16 (ACT, fused scale).
      The DVE keeps exact f32 running colsums of the exp chunks; the
      cross-partition sum is then 2 fp32 ones-matmuls (one per q-half,
      replicating the per-query sum on all 128 partitions -> full-width
      reciprocal) instead of 32 bf16 ones-matmuls. Those 2 matmuls are
      deferred past the first G chain so the PE never waits on the DVE
      add tail.
  B2: G[d,q] = xn^T @ expT ; Gn = G * recip (DVE, PSUM drain)
      ytT[o,q] = W2T^T @ Gn -> DRAM
512-moving matmuls through a 6-bank PSUM rotation (+2 banks for the
colsum matmuls); 1024-moving fails walrus codegen (s3d3_mm_num_elements).
Weight loads are split across the Sync and Scalar HWDGE queues so chunk
pairs land 2x faster; wv/wo reuse wq/wk's SBUF bytes (zone-gated on M's
last matmul, landing before W2T needs them). Measured: 240549ns HW exec
(vs 327390ns baseline), rel err 7.32e-3. Runs on a thermally-throttled
chip (P0, PE at 2.0GHz instead of 2.4) measure ~18% slower.
Biases are zeros by problem spec; bo is applied on host if nonzero.
"""
import sys

if '/opt/trn_rl_repo' not in sys.path:
    sys.path.insert(0, '/opt/trn_rl_repo')

from contextlib import ExitStack

import numpy as np
import ml_dtypes

import concourse.bacc as bacc_mod
import concourse.mybir as mybir
import concourse.tile as tile
from concourse.bass_utils import run_bass_kernel_spmd

F32 = mybir.dt.float32
BF16 = mybir.dt.bfloat16
EXP = mybir.ActivationFunctionType.Exp
MULT = mybir.AluOpType.mult
ADD = mybir.AluOpType.add

B, S, D = 4, 2048, 1024
SQ = 1024           # queries per core
P = 128
NDC = D // P        # 8 chunks of 128 over d / d' / e / o
NKC = S // P        # 16 key chunks
NQH = SQ // 512     # 2 query column-halves (moving dim 512)
NH2 = D // 512      # 2 column-halves of a [*, D] product

LAST_RESULT = [None]
_CACHE = {}


def build_nc():
    nc = bacc_mod.Bacc("TRN2", target_bir_lowering=False, debug=False)

    xt = nc.dram_tensor("xt", [D, S], BF16, kind="ExternalInput")
    xn = nc.dram_tensor("xn", [S, D], BF16, kind="ExternalInput")
    wq = nc.dram_tensor("wq", [D, D], BF16, kind="ExternalInput")
    wk = nc.dram_tensor("wk", [D, D], BF16, kind="ExternalInput")
    wv = nc.dram_tensor("wv", [D, D], BF16, kind="ExternalInput")
    wot = nc.dram_tensor("wot", [D, D], BF16, kind="ExternalInput")
    yt = nc.dram_tensor("yt", [D, SQ], F32, kind="ExternalOutput")

    def part3(ap):  # [R, C] dram -> [128, R/128, C] (rows on partitions)
        return ap.rearrange("(o i) c -> i o c", i=P)

    with tile.TileContext(nc) as tc, ExitStack() as ctx:
        pers = ctx.enter_context(tc.tile_pool(name="pers", bufs=1))
        ones_bf = pers.tile([P, P], BF16)
        nc.vector.memset(ones_bf[:], 1.0)
        bcast_sb = pers.tile([P, SQ], F32)
        acc_sb = pers.tile([P, SQ], F32)   # per-partition partial colsums

        # 6-bank PSUM rotation shared by every accumulation chain; +2 banks
        # for the two q-halves' colsum accumulators during B1
        mps = ctx.enter_context(tc.tile_pool(name="mps", bufs=6, space="PSUM"))

        # persistents: W2T, x^T, Zq, G (left); expT, xn (right)
        w2tp = ctx.enter_context(tc.tile_pool(name="w2t", bufs=1))
        w2t_sb = w2tp.tile([P, NDC, D], BF16)      # 16 KB/part
        xtp = ctx.enter_context(tc.tile_pool(name="xtp", bufs=1))
        xt_sb = xtp.tile([P, NDC, S], BF16)        # 32 KB/part
        zqp = ctx.enter_context(tc.tile_pool(name="zqp", bufs=1))
        zq_sb = zqp.tile([P, NDC, SQ], BF16)       # 16 KB/part
        gp = ctx.enter_context(tc.tile_pool(name="gp", bufs=1))
        g_sb = gp.tile([P, NDC, SQ], BF16)         # 16 KB/part
        yp = ctx.enter_context(tc.tile_pool(name="yp", bufs=3))
        epool = ctx.enter_context(
            tc.tile_pool(name="expt", bufs=1, side="right"))
        expt_sb = epool.tile([P, NKC, SQ], BF16)   # 32 KB/part
        xnp = ctx.enter_context(
            tc.tile_pool(name="xnp", bufs=1, side="right"))
        xn_sb = xnp.tile([P, NKC, D], BF16)        # 32 KB/part

        # ~4.3us of dummy matmuls on ones_bf while the first weight chunks
        # stream in: keeps the PE busy from ~8us so the HAM throttle reaches
        # K=8/8 before the first real matmul
        ps_w = mps.tile([P, 512], F32, tag="ps", name="warm")
        for i in range(40):
            nc.tensor.matmul(ps_w[:, 0:P], ones_bf[:], ones_bf[:],
                             start=(i == 0), stop=(i == 39))
        nc.vector.tensor_copy(acc_sb[:, 0:P], ps_w[:, 0:P])  # dead store

        def chain_waves(chains, lhs_of, rhs_of, out_of, nacc, tagbase):
            # waves of 6 chains; acc-step outer so 6 independent PSUM
            # accumulations ride out chunked-DMA arrival
            for w0 in range(0, len(chains), 6):
                wave = chains[w0:w0 + 6]
                ps = [mps.tile([P, 512], F32, tag="ps",
                               name=f"{tagbase}{w0}_{i}")
                      for i in range(len(wave))]
                for a in range(nacc):
                    for i, ch in enumerate(wave):
                        nc.tensor.matmul(ps[i][:], lhs_of(ch, a),
                                         rhs_of(ch, a),
                                         start=(a == 0), stop=(a == nacc - 1))
                for i, ch in enumerate(wave):
                    nc.vector.tensor_copy(out_of(ch), ps[i][:])

        chains = [(dc, h2) for dc in range(NDC) for h2 in range(NH2)]

        with tc.tile_pool(name="mp", bufs=1) as mp:
            m_sb = mp.tile([P, NDC, D], BF16)      # 16 KB/part
            # ---- A1: M = Wq^T Wk ----
            with tc.tile_pool(name="wqk", bufs=1) as wqk:
                wq_sb = wqk.tile([P, NDC, D], BF16)
                wk_sb = wqk.tile([P, NDC, D], BF16)
                # wq on sync, wk on scalar so chunk pairs land 2x faster;
                # chunk 0 split in half-columns so the first matmul's
                # operands arrive ~0.5us earlier
                for h in range(2):
                    nc.sync.dma_start(wq_sb[:, 0, h * 512:(h + 1) * 512],
                                      wq[0:P, h * 512:(h + 1) * 512])
                    nc.scalar.dma_start(wk_sb[:, 0, h * 512:(h + 1) * 512],
                                        wk[0:P, h * 512:(h + 1) * 512])
                for c in range(1, NDC):
                    nc.sync.dma_start(wq_sb[:, c, :],
                                      wq[c * P:(c + 1) * P, :])
                    nc.scalar.dma_start(wk_sb[:, c, :],
                                        wk[c * P:(c + 1) * P, :])
                # x^T and xn stream behind the weights on both queues
                for sh in range(2):
                    nc.sync.dma_start(
                        xt_sb[:, :, sh * 1024:sh * 1024 + 512],
                        part3(xt[:, sh * 1024:sh * 1024 + 512]))
                    nc.scalar.dma_start(
                        xt_sb[:, :, sh * 1024 + 512:(sh + 1) * 1024],
                        part3(xt[:, sh * 1024 + 512:(sh + 1) * 1024]))
                nc.sync.dma_start(xn_sb[:], part3(xn))
                # M[d, d'] = sum_e1 Wq[e1, d] Wk[e1, d']
                chain_waves(
                    chains,
                    lambda ch, a: wq_sb[:, a, ch[0] * P:(ch[0] + 1) * P],
                    lambda ch, a: wk_sb[:, a, ch[1] * 512:(ch[1] + 1) * 512],
                    lambda ch: m_sb[:, ch[0], ch[1] * 512:(ch[1] + 1) * 512],
                    NDC, "mm")

            # ---- A2: Zq = M^T @ xq (xq = xt cols 0:1024) ----
            chain_waves(
                chains,
                lambda ch, a: m_sb[:, a, ch[0] * P:(ch[0] + 1) * P],
                lambda ch, a: xt_sb[:, a, ch[1] * 512:(ch[1] + 1) * 512],
                lambda ch: zq_sb[:, ch[0], ch[1] * 512:(ch[1] + 1) * 512],
                NDC, "zq")

            # ---- A3: W2T = Wv^T Wo^T (wv/wo reuse wq/wk's bytes; DMA is
            # zone-gated on M's last matmul, landing before W2T starts) ----
            with tc.tile_pool(name="wvo", bufs=1) as wvo:
                wv_sb = wvo.tile([P, NDC, D], BF16)
                wo_sb = wvo.tile([P, NDC, D], BF16)
                for c in range(NDC):
                    nc.sync.dma_start(wv_sb[:, c, :],
                                      wv[c * P:(c + 1) * P, :])
                    nc.scalar.dma_start(wo_sb[:, c, :],
                                        wot[c * P:(c + 1) * P, :])
                # W2T[d, o] = sum_e Wv[e, d] Wo[o, e]
                chain_waves(
                    chains,
                    lambda ch, a: wv_sb[:, a, ch[0] * P:(ch[0] + 1) * P],
                    lambda ch, a: wo_sb[:, a, ch[1] * 512:(ch[1] + 1) * 512],
                    lambda ch: w2t_sb[:, ch[0],
                                      ch[1] * 512:(ch[1] + 1) * 512],
                    NDC, "w2")

        # ---- B1: scoresT -> expT; DVE keeps per-partition running colsums
        # (f32, exact) so the PE does 2 fp32 ones-matmuls instead of 32 ----
        with tc.tile_pool(name="sump", bufs=2, space="PSUM") as sump:
            for qh in range(NQH):
                q0 = qh * 512
                for kc in range(NKC):
                    ps_s = mps.tile([P, 512], F32, tag="ps",
                                    name=f"pss{qh}_{kc}")
                    for dc in range(NDC):
                        nc.tensor.matmul(
                            ps_s[:], xt_sb[:, dc, kc * P:(kc + 1) * P],
                            zq_sb[:, dc, q0:q0 + 512],
                            start=(dc == 0), stop=(dc == NDC - 1))
                    nc.scalar.activation(
                        expt_sb[:, kc, q0:q0 + 512], ps_s[:], EXP,
                        scale=1.0 / 32.0)
                    if kc == 0:
                        nc.vector.tensor_copy(acc_sb[:, q0:q0 + 512],
                                              expt_sb[:, 0, q0:q0 + 512])
                    else:
                        nc.vector.tensor_tensor(
                            acc_sb[:, q0:q0 + 512], acc_sb[:, q0:q0 + 512],
                            expt_sb[:, kc, q0:q0 + 512], ADD)

            # first G chain runs before the colsum matmuls so the PE never
            # waits on the tail of the DVE add chain
            ps_g0 = mps.tile([P, 512], F32, tag="ps", name="pg0_0")
            for kc in range(NKC):
                nc.tensor.matmul(
                    ps_g0[:], xn_sb[:, kc, 0:P], expt_sb[:, kc, 0:512],
                    start=(kc == 0), stop=(kc == NKC - 1))
            # cross-partition sum + broadcast in one bf16 ones-matmul per
            # q-half -> full-width reciprocal. The bf16 casts land in
            # zq_sb (dead after B1) and are both emitted before any
            # reciprocal: the DVE is serial, and a cast queued behind a
            # 4us reciprocal would stall the second ones-matmul
            for qh in range(NQH):
                nc.vector.tensor_copy(zq_sb[:, qh, 0:512],
                                      acc_sb[:, qh * 512:qh * 512 + 512])
            for qh in range(NQH):
                q0 = qh * 512
                ps_sum = sump.tile([P, 512], F32, tag="pssum")
                nc.tensor.matmul(ps_sum[:], ones_bf[:], zq_sb[:, qh, 0:512],
                                 start=True, stop=True)
                nc.vector.reciprocal(bcast_sb[:, q0:q0 + 512], ps_sum[:])
            nc.vector.tensor_tensor(
                g_sb[:, 0, 0:512], ps_g0[:], bcast_sb[:, 0:512], MULT)

        # ---- B2: G = xn^T @ expT, normalized; ytT = W2T^T @ Gn ----
        for qh in range(NQH):
            q0 = qh * 512
            for dc in range(NDC):
                if qh == 0 and dc == 0:
                    continue  # already issued above
                ps_g = mps.tile([P, 512], F32, tag="ps", name=f"pg{qh}_{dc}")
                for kc in range(NKC):
                    nc.tensor.matmul(
                        ps_g[:], xn_sb[:, kc, dc * P:(dc + 1) * P],
                        expt_sb[:, kc, q0:q0 + 512],
                        start=(kc == 0), stop=(kc == NKC - 1))
                nc.vector.tensor_tensor(
                    g_sb[:, dc, q0:q0 + 512], ps_g[:],
                    bcast_sb[:, q0:q0 + 512], MULT)
        for qh in range(NQH):
            q0 = qh * 512
            for oc in range(NDC):
                ps_y = mps.tile([P, 512], F32, tag="ps", name=f"py{qh}_{oc}")
                for dc in range(NDC):
                    nc.tensor.matmul(
                        ps_y[:], w2t_sb[:, dc, oc * P:(oc + 1) * P],
                        g_sb[:, dc, q0:q0 + 512],
                        start=(dc == 0), stop=(dc == NDC - 1))
                yst = yp.tile([P, 512], F32, tag="yst")
                if qh == NQH - 1 and oc == NDC - 1:
                    # final drain split in half so the last store starts
                    # while the second half-copy runs
                    for h in range(2):
                        h0 = h * 256
                        nc.vector.tensor_copy(yst[:, h0:h0 + 256],
                                              ps_y[:, h0:h0 + 256])
                        nc.scalar.dma_start(
                            yt[oc * P:(oc + 1) * P, q0 + h0:q0 + h0 + 256],
                            yst[:, h0:h0 + 256])
                else:
                    nc.vector.tensor_copy(yst[:], ps_y[:])
                    nc.scalar.dma_start(
                        yt[oc * P:(oc + 1) * P, q0:q0 + 512], yst[:])

    nc.compile()
    return nc


def _get_nc():
    if "nc" not in _CACHE:
        _CACHE["nc"] = build_nc()
    return _CACHE["nc"]


def kernel(x, Wq, bq, Wk, bk, Wv, bv, Wo, bo, _trace=False):
    x = np.asarray(x, dtype=np.float32)
    bf = ml_dtypes.bfloat16
    wq_b = np.ascontiguousarray(np.asarray(Wq, dtype=np.float32)).astype(bf)
    wk_b = np.ascontiguousarray(np.asarray(Wk, dtype=np.float32)).astype(bf)
    wv_b = np.ascontiguousarray(np.asarray(Wv, dtype=np.float32)).astype(bf)
    wot_b = np.ascontiguousarray(
        np.asarray(Wo, dtype=np.float32).T).astype(bf)

    in_maps = []
    for c in range(8):
        b, h = c // 2, c % 2
        xb = x[b]
        if h == 0:
            xroll = xb
        else:
            xroll = np.concatenate([xb[SQ:], xb[:SQ]], axis=0)
        xtb_ = np.ascontiguousarray(xroll.T).astype(bf)
        xnb = np.ascontiguousarray(xroll).astype(bf)
        in_maps.append({"xt": xtb_, "xn": xnb, "wq": wq_b, "wk": wk_b,
                        "wv": wv_b, "wot": wot_b})

    nc = _get_nc()
    kw = {}
    if _trace:
        kw = dict(trace=True, stitch_traces=False)
    res = run_bass_kernel_spmd(nc, in_maps, core_ids=list(range(8)), **kw)
    LAST_RESULT[0] = res

    y = np.empty((B, S, D), dtype=np.float32)
    for c in range(8):
        b, h = c // 2, c % 2
        y[b, h * SQ:(h + 1) * SQ, :] = res.results[c]["yt"].T

    bo = np.asarray(bo, dtype=np.float32)
    if bo.any():
        y = y + bo
    return y
